# revision 9
# baseline (speedup 1.0000x reference)
"""Causal self-attention Bass/Trainium2 kernel.

Problem: B=4, T=2048, D=768, NH=12 heads (dh=64), fp32 I/O.

Sharding (8 NeuronCores, no collectives):
  core = b * 2 + hg  for batch b in 0..3, head-group hg in 0..1.
  Each core computes 6 heads (hg*6 .. hg*6+5) of one batch:
    Q/K/V projections for its heads, causal attention, and the partial
    output projection y_part = Z_part @ Wo_part (row-split contraction).
  Host sums the two partial outputs per batch and adds bo.

Per-core kernel layout (everything transposed so the contraction dim is
on partitions; host pre-transposes, which is free):
  xT  [768, 2048]          QT/KT [384, 2048] (pairs of heads per 128-row tile)
  V'  [2048, 6*65]         (ones column appended per head -> softmax sums)
  S^T [128k, 512q] blocks, P = exp(S/8) (no max subtraction: |logits| < 10),
  O'  = V'.T @ P^T accumulated over k tiles -> row 64 holds softmax sums.
  Normalize by broadcasting 1/sums, assemble Z^T, then y^T = Wo_sel @ Z.
"""

import numpy as np
import ml_dtypes

import concourse.bass as bass
from concourse import bacc
import concourse.mybir as mybir
import concourse.tile as tile
from concourse.bass_utils import run_bass_kernel_spmd

B, T, D, NH, DH = 4, 2048, 768, 12, 64
HPC = 6          # heads per core
NPAIR = 3        # head pairs per core
TQ = 512         # query tile (free dim of S^T blocks)
NQT = T // TQ    # 4
TKB = 128        # key tile (partition dim of S^T blocks)
NKT = T // TKB   # 16
KD = D // 128    # 6 contraction tiles for the projections
VW = DH + 1      # 65: V plus ones column

# Matmul/storage dtype: "bf16", "fp32", or "fp32r".
MM_MODE = "bf16"

_f32 = mybir.dt.float32


def _dts():
    if MM_MODE == "bf16":
        return mybir.dt.bfloat16, ml_dtypes.bfloat16, False
    return _f32, np.float32, (MM_MODE == "fp32r")


def _build_program():
    ST_DT, _, use_r = _dts()

    def mm(ap):
        return ap.bitcast(mybir.dt.float32r) if use_r else ap

    nc = bacc.Bacc()
    xT_d = nc.dram_tensor("xT", [KD, 128, T], ST_DT, kind="ExternalInput")
    wq_d = nc.dram_tensor("wqT", [KD, 128, HPC * DH], ST_DT, kind="ExternalInput")
    wk_d = nc.dram_tensor("wkT", [KD, 128, HPC * DH], ST_DT, kind="ExternalInput")
    wv_d = nc.dram_tensor("wvT", [KD, 128, HPC * DH], ST_DT, kind="ExternalInput")
    wo_d = nc.dram_tensor("woT", [NPAIR, 128, D], ST_DT, kind="ExternalInput")
    bq_d = nc.dram_tensor("bqT", [128, NPAIR], _f32, kind="ExternalInput")
    bk_d = nc.dram_tensor("bkT", [128, NPAIR], _f32, kind="ExternalInput")
    bvb_d = nc.dram_tensor("bvb", [HPC * VW], _f32, kind="ExternalInput")
    # DRAM scratch used to broadcast the per-column softmax sums across
    # partitions (SBUF->DRAM->stride-0 DMA back; DVE cannot cross partitions)
    scr_d = nc.dram_tensor("rscratch", [NPAIR, NQT, 2, TQ], _f32, kind="Internal")
    yT_d = nc.dram_tensor("yT", [KD, 128, T], _f32, kind="ExternalOutput")

    with tile.TileContext(nc) as tc:
        with (
            tc.tile_pool(name="const", bufs=1) as const,
            tc.tile_pool(name="ptp", bufs=3) as ptp,
            tc.tile_pool(name="workp", bufs=3) as workp,
            tc.tile_pool(name="ps512", bufs=4, space="PSUM") as ps512,
            tc.tile_pool(name="ps1024", bufs=2, space="PSUM") as ps1024,
        ):
            # ---- constants / persistent tensors ----
            xT_sb = const.tile([128, KD, T], ST_DT)
            wq_sb = const.tile([128, KD, HPC * DH], ST_DT)
            wk_sb = const.tile([128, KD, HPC * DH], ST_DT)
            wv_sb = const.tile([128, KD, HPC * DH], ST_DT)
            wo_sb = const.tile([128, NPAIR, D], ST_DT)
            bq_sb = const.tile([128, NPAIR], _f32)
            bk_sb = const.tile([128, NPAIR], _f32)
            bvb_sb = const.tile([128, HPC * VW], _f32)
            qt_sb = const.tile([128, NPAIR, T], ST_DT)
            kt_sb = const.tile([128, NPAIR, T], ST_DT)
            v_sb = const.tile([128, NKT, HPC * VW], ST_DT)
            zt_sb = const.tile([128, NPAIR, T], ST_DT)

            for kt in range(KD):
                nc.sync.dma_start(out=xT_sb[:, kt, :], in_=xT_d[kt])
                nc.sync.dma_start(out=wq_sb[:, kt, :], in_=wq_d[kt])
                nc.sync.dma_start(out=wk_sb[:, kt, :], in_=wk_d[kt])
                nc.sync.dma_start(out=wv_sb[:, kt, :], in_=wv_d[kt])
            for kt in range(NPAIR):
                nc.sync.dma_start(out=wo_sb[:, kt, :], in_=wo_d[kt])
            nc.sync.dma_start(out=bq_sb, in_=bq_d[:, :])
            nc.sync.dma_start(out=bk_sb, in_=bk_d[:, :])
            bvb_ap = bvb_d[:]
            bvb_bcast = bass.AP(
                tensor=bvb_ap.tensor, offset=bvb_ap.offset,
                ap=[[0, 128]] + list(bvb_ap.ap),
            )
            nc.gpsimd.dma_start(out=bvb_sb, in_=bvb_bcast)

            # ones column of V' (softmax denominator accumulator)
            v_by_head = v_sb.rearrange("p m (h c) -> p m h c", c=VW)
            nc.vector.memset(v_by_head[:, :, :, DH:VW], 1.0)

            # causal masks for the 4 diagonal offsets: keep q >= k
            masks = []
            for i in range(4):
                mask_t = const.tile([128, TQ], ST_DT, name=f"mask{i}")
                nc.vector.memset(mask_t, 1.0)
                nc.gpsimd.affine_select(
                    out=mask_t,
                    in_=mask_t,
                    compare_op=mybir.AluOpType.is_ge,
                    fill=0.0,
                    base=-(i * TKB),
                    pattern=[[1, TQ]],
                    channel_multiplier=-1,
                )
                masks.append(mask_t)

            # ---- Q/K projections: dest^T[m, t] = W_sel @ x^T (+bias) ----
            for w_sb, b_sb, dest in ((wq_sb, bq_sb, qt_sb), (wk_sb, bk_sb, kt_sb)):
                for mt in range(NPAIR):
                    pss = []
                    for nt in range(NQT):
                        ps_qk = ps512.tile([128, TQ], _f32, tag="ps512", name=f"psqk{nt}")
                        pss.append(ps_qk)
                    for kt in range(KD):
                        for nt in range(NQT):
                            nc.tensor.matmul(
                                pss[nt],
                                lhsT=mm(w_sb[:, kt, mt * 128 : (mt + 1) * 128]),
                                rhs=mm(xT_sb[:, kt, nt * TQ : (nt + 1) * TQ]),
                                start=(kt == 0),
                                stop=(kt == KD - 1),
                            )
                    for nt in range(NQT):
                        nc.vector.tensor_tensor(
                            out=dest[:, mt, nt * TQ : (nt + 1) * TQ],
                            in0=pss[nt],
                            in1=b_sb[:, mt : mt + 1].to_broadcast((128, TQ)),
                            op=mybir.AluOpType.add,
                        )

            # ---- V projection (per pair): V[t, hd] = x @ Wv_sel^T + bv ----
            for mt in range(NKT):
                for p in range(NPAIR):
                    psv = ps512.tile([128, 128], _f32, tag="ps512", name="psv")
                    for kt in range(KD):
                        nc.tensor.matmul(
                            psv,
                            lhsT=mm(xT_sb[:, kt, mt * 128 : (mt + 1) * 128]),
                            rhs=mm(wv_sb[:, kt, p * 128 : (p + 1) * 128]),
                            start=(kt == 0),
                            stop=(kt == KD - 1),
                        )
                    dst = v_by_head[:, mt, 2 * p : 2 * p + 2, 0:DH]
                    bvv = bvb_sb.rearrange("p (h c) -> p h c", c=VW)[
                        :, 2 * p : 2 * p + 2, 0:DH
                    ]
                    nc.vector.tensor_tensor(
                        out=dst,
                        in0=psv.rearrange("p (h c) -> p h c", c=DH),
                        in1=bvv,
                        op=mybir.AluOpType.add,
                    )

            # ---- attention per head pair ----
            for p in range(NPAIR):
                qA = qt_sb[0:64, p, :]
                qB = qt_sb[64:128, p, :]
                kA = kt_sb[0:64, p, :]
                kB = kt_sb[64:128, p, :]
                for qt in range(NQT):
                    nk = 4 * (qt + 1)
                    oA = ps512.tile([128, TQ], _f32, tag="ps512", name="oA")
                    oB = ps512.tile([128, TQ], _f32, tag="ps512", name="oB")
                    qsl = slice(qt * TQ, (qt + 1) * TQ)
                    pts = [None] * nk

                    def emit_qk(kt):
                        sab = ps1024.tile([128, 2, TQ], _f32, tag="sab", name="sab")
                        ksl = slice(kt * TKB, (kt + 1) * TKB)
                        nc.tensor.matmul(
                            sab[:, 0, :], lhsT=mm(kA[:, ksl]), rhs=mm(qA[:, qsl]),
                            start=True, stop=True,
                        )
                        nc.tensor.matmul(
                            sab[:, 1, :], lhsT=mm(kB[:, ksl]), rhs=mm(qB[:, qsl]),
                            start=True, stop=True,
                        )
                        pt = ptp.tile([128, 2, TQ], ST_DT, tag="pt", name="pt")
                        nc.scalar.activation(
                            out=pt, in_=sab,
                            func=mybir.ActivationFunctionType.Exp,
                            scale=0.125,
                        )
                        if kt >= 4 * qt:  # diagonal block: apply causal mask
                            mask_t = masks[kt - 4 * qt]
                            nc.vector.tensor_mul(pt[:, 0, :], pt[:, 0, :], mask_t)
                            nc.vector.tensor_mul(pt[:, 1, :], pt[:, 1, :], mask_t)
                        pts[kt] = pt

                    def emit_pv(kt):
                        st, sp = (kt == 0), (kt == nk - 1)
                        pt = pts[kt]
                        nc.tensor.matmul(
                            oA[0:VW, :],
                            lhsT=mm(v_sb[:, kt, (2 * p) * VW : (2 * p + 1) * VW]),
                            rhs=mm(pt[:, 0, :]),
                            start=st, stop=sp,
                        )
                        nc.tensor.matmul(
                            oB[0:VW, :],
                            lhsT=mm(v_sb[:, kt, (2 * p + 1) * VW : (2 * p + 2) * VW]),
                            rhs=mm(pt[:, 1, :]),
                            start=st, stop=sp,
                        )
                        pts[kt] = None

                    # software pipeline: PE stays one block ahead of ACT
                    for kt in range(nk):
                        emit_qk(kt)
                        if kt >= 1:
                            emit_pv(kt - 1)
                    emit_pv(nk - 1)

                    # normalize by the accumulated softmax sums (row 64)
                    recA = workp.tile([65, TQ], _f32, tag="recA", name="recA")
                    recB = workp.tile([65, TQ], _f32, tag="recB", name="recB")
                    nc.vector.reciprocal(out=recA[64:65, :], in_=oA[64:65, :])
                    nc.vector.reciprocal(out=recB[64:65, :], in_=oB[64:65, :])
                    rbA = workp.tile([64, TQ], _f32, tag="rbA", name="rbA")
                    rbB = workp.tile([64, TQ], _f32, tag="rbB", name="rbB")
                    nc.sync.dma_start(out=scr_d[p, qt, 0, :], in_=recA[64:65, :])
                    nc.sync.dma_start(out=scr_d[p, qt, 1, :], in_=recB[64:65, :])
                    sA = scr_d[p, qt, 0, :]
                    sB = scr_d[p, qt, 1, :]
                    nc.sync.dma_start(out=rbA, in_=bass.AP(
                        tensor=sA.tensor, offset=sA.offset,
                        ap=[[0, 64]] + list(sA.ap)))
                    nc.sync.dma_start(out=rbB, in_=bass.AP(
                        tensor=sB.tensor, offset=sB.offset,
                        ap=[[0, 64]] + list(sB.ap)))
                    nc.vector.tensor_mul(zt_sb[0:64, p, qsl], oA[0:64, :], rbA)
                    ztmp = workp.tile([64, TQ], ST_DT, tag="ztmp", name="ztmp")
                    nc.vector.tensor_mul(ztmp, oB[0:64, :], rbB)
                    nc.sync.dma_start(out=zt_sb[64:128, p, qsl], in_=ztmp)

            # ---- output projection: y^T[o, t] = Wo_sel^T... @ Z ----
            for mt in range(KD):
                psy = []
                for nt in range(NQT):
                    ps_y = ps512.tile([128, TQ], _f32, tag="ps512", name=f"psy{nt}")
                    psy.append(ps_y)
                for kt in range(NPAIR):
                    for nt in range(NQT):
                        nc.tensor.matmul(
                            psy[nt],
                            lhsT=mm(wo_sb[:, kt, mt * 128 : (mt + 1) * 128]),
                            rhs=mm(zt_sb[:, kt, nt * TQ : (nt + 1) * TQ]),
                            start=(kt == 0),
                            stop=(kt == NPAIR - 1),
                        )
                for nt in range(NQT):
                    yt = workp.tile([128, TQ], _f32, tag="yt", name="yt")
                    nc.vector.tensor_copy(yt, psy[nt])
                    nc.sync.dma_start(
                        out=yT_d[mt, :, nt * TQ : (nt + 1) * TQ], in_=yt
                    )

    if not nc.is_finalized():
        nc.finalize()
    return nc


_CACHE = {}


def get_program():
    key = MM_MODE
    if key not in _CACHE:
        _CACHE[key] = _build_program()
    return _CACHE[key]


def make_in_maps(x, wq, bq, wk, bk, wv, bv, wo, bo):
    _, np_dt, _ = _dts()
    x, wq, bq, wk, bk, wv, bv, wo, bo = (
        np.asarray(a, dtype=np.float32) for a in (x, wq, bq, wk, bk, wv, bv, wo, bo)
    )
    in_maps = []
    for core in range(8):
        b, hg = core // 2, core % 2
        sl = slice(hg * HPC * DH, (hg + 1) * HPC * DH)
        xT = np.ascontiguousarray(x[b].T).astype(np_dt).reshape(KD, 128, T)
        wqT = np.ascontiguousarray(wq[sl, :].T).astype(np_dt).reshape(KD, 128, HPC * DH)
        wkT = np.ascontiguousarray(wk[sl, :].T).astype(np_dt).reshape(KD, 128, HPC * DH)
        wvT = np.ascontiguousarray(wv[sl, :].T).astype(np_dt).reshape(KD, 128, HPC * DH)
        woT = np.ascontiguousarray(wo[:, sl].T).astype(np_dt).reshape(NPAIR, 128, D)
        bqT = np.ascontiguousarray(bq[sl].reshape(NPAIR, 128).T)
        bkT = np.ascontiguousarray(bk[sl].reshape(NPAIR, 128).T)
        bvb = np.zeros((HPC, VW), np.float32)
        bvb[:, :DH] = bv[sl].reshape(HPC, DH)
        bvb[:, DH] = 1.0
        in_maps.append(
            dict(xT=xT, wqT=wqT, wkT=wkT, wvT=wvT, woT=woT,
                 bqT=bqT, bkT=bkT, bvb=bvb.reshape(-1))
        )
    return in_maps


def assemble_output(results, bo):
    y = np.zeros((B, T, D), np.float32)
    for core in range(8):
        y[core // 2] += results[core]["yT"].reshape(D, T).T
    y += np.asarray(bo, np.float32)[None, None, :]
    return y


def kernel(**inputs):
    nc = get_program()
    in_maps = make_in_maps(**inputs)
    res = run_bass_kernel_spmd(nc, in_maps, core_ids=list(range(8)))
    return assemble_output(res.results, inputs["bo"])


if __name__ == "__main__":
    nc = get_program()
    print("program built OK")


# revision 13
# speedup vs baseline: 1.0851x; 1.0851x over previous
"""Causal self-attention Bass/Trainium2 kernel.

Problem: B=4, T=2048, D=768, NH=12 heads (dh=64), fp32 I/O.

Sharding (8 NeuronCores, no collectives):
  core = b * 2 + hg  for batch b in 0..3, head-group hg in 0..1.
  Each core computes 6 heads (hg*6 .. hg*6+5) of one batch:
    Q/K/V projections for its heads, causal attention, and the partial
    output projection y_part = Z_part @ Wo_part (row-split contraction).
  Host sums the two partial outputs per batch and adds bo.

Per-core kernel layout (everything transposed so the contraction dim is
on partitions; host pre-transposes, which is free):
  xT  [768, 2048]          QT/KT [384, 2048] (pairs of heads per 128-row tile)
  V'  [2048, 6*65]         (ones column appended per head -> softmax sums)
  S^T [128k, 512q] blocks, P = exp(S/8) (no max subtraction: |logits| < 10),
  O'  = V'.T @ P^T accumulated over k tiles -> row 64 holds softmax sums.
  Normalize by broadcasting 1/sums, assemble Z^T, then y^T = Wo_sel @ Z.
"""

import numpy as np
import ml_dtypes

import concourse.bass as bass
from concourse import bacc
import concourse.mybir as mybir
import concourse.tile as tile
from concourse.bass_utils import run_bass_kernel_spmd

B, T, D, NH, DH = 4, 2048, 768, 12, 64
HPC = 6          # heads per core
NPAIR = 3        # head pairs per core
TQ = 512         # query tile (free dim of S^T blocks)
NQT = T // TQ    # 4
TKB = 128        # key tile (partition dim of S^T blocks)
NKT = T // TKB   # 16
KD = D // 128    # 6 contraction tiles for the projections
VW = DH + 1      # 65: V plus ones column

# Matmul/storage dtype: "bf16", "fp32", or "fp32r".
MM_MODE = "bf16"

_f32 = mybir.dt.float32


def _dts():
    if MM_MODE == "bf16":
        return mybir.dt.bfloat16, ml_dtypes.bfloat16, False
    return _f32, np.float32, (MM_MODE == "fp32r")


def _build_program():
    ST_DT, _, use_r = _dts()

    def mm(ap):
        return ap.bitcast(mybir.dt.float32r) if use_r else ap

    nc = bacc.Bacc()
    xT_d = nc.dram_tensor("xT", [KD, 128, T], ST_DT, kind="ExternalInput")
    wq_d = nc.dram_tensor("wqT", [KD, 128, HPC * DH], ST_DT, kind="ExternalInput")
    wk_d = nc.dram_tensor("wkT", [KD, 128, HPC * DH], ST_DT, kind="ExternalInput")
    wv_d = nc.dram_tensor("wvT", [KD, 128, HPC * DH], ST_DT, kind="ExternalInput")
    wo_d = nc.dram_tensor("woT", [NPAIR, 128, D], ST_DT, kind="ExternalInput")
    bq_d = nc.dram_tensor("bqT", [128, NPAIR], _f32, kind="ExternalInput")
    bk_d = nc.dram_tensor("bkT", [128, NPAIR], _f32, kind="ExternalInput")
    bvb_d = nc.dram_tensor("bvb", [HPC * VW], _f32, kind="ExternalInput")
    # DRAM scratch used to broadcast the per-column softmax sums across
    # partitions (SBUF->DRAM->stride-0 DMA back; DVE cannot cross partitions)
    scr_d = nc.dram_tensor("rscratch", [NPAIR, NQT, 2, TQ], _f32, kind="Internal")
    scr2_d = nc.dram_tensor("rscratch2", [NPAIR, NQT, 2, TQ], _f32, kind="Internal")
    yT_d = nc.dram_tensor("yT", [KD, 128, T], _f32, kind="ExternalOutput")

    with tile.TileContext(nc) as tc:
        with (
            tc.tile_pool(name="const", bufs=1) as const,
            tc.tile_pool(name="ptp", bufs=3) as ptp,
            tc.tile_pool(name="workp", bufs=3) as workp,
            tc.tile_pool(name="ps512", bufs=4, space="PSUM") as ps512,
            tc.tile_pool(name="ps1024", bufs=2, space="PSUM") as ps1024,
        ):
            # ---- constants / persistent tensors ----
            xT_sb = const.tile([128, KD, T], ST_DT)
            wq_sb = const.tile([128, KD, HPC * DH], ST_DT)
            wk_sb = const.tile([128, KD, HPC * DH], ST_DT)
            wv_sb = const.tile([128, KD, HPC * DH], ST_DT)
            wo_sb = const.tile([128, NPAIR, D], ST_DT)
            bq_sb = const.tile([128, NPAIR], _f32)
            bk_sb = const.tile([128, NPAIR], _f32)
            bvb_sb = const.tile([128, HPC * VW], _f32)
            qt_sb = const.tile([128, NPAIR, T], ST_DT)
            kt_sb = const.tile([128, NPAIR, T], ST_DT)
            v_sb = const.tile([128, NKT, HPC * VW], ST_DT)
            zt_sb = const.tile([128, NPAIR, T], ST_DT)

            for kt in range(KD):
                # split xT tiles into 512-col chunks for finer DMA pipelining
                for c in range(4):
                    nc.sync.dma_start(
                        out=xT_sb[:, kt, c * TQ : (c + 1) * TQ],
                        in_=xT_d[kt][:, c * TQ : (c + 1) * TQ],
                    )
                nc.sync.dma_start(out=wq_sb[:, kt, :], in_=wq_d[kt])
                nc.sync.dma_start(out=wk_sb[:, kt, :], in_=wk_d[kt])
                nc.sync.dma_start(out=wv_sb[:, kt, :], in_=wv_d[kt])
            for kt in range(NPAIR):
                nc.sync.dma_start(out=wo_sb[:, kt, :], in_=wo_d[kt])
            nc.sync.dma_start(out=bq_sb, in_=bq_d[:, :])
            nc.sync.dma_start(out=bk_sb, in_=bk_d[:, :])
            bvb_ap = bvb_d[:]
            bvb_bcast = bass.AP(
                tensor=bvb_ap.tensor, offset=bvb_ap.offset,
                ap=[[0, 128]] + list(bvb_ap.ap),
            )
            nc.gpsimd.dma_start(out=bvb_sb, in_=bvb_bcast)

            # ones column of V' (softmax denominator accumulator)
            v_by_head = v_sb.rearrange("p m (h c) -> p m h c", c=VW)
            nc.vector.memset(v_by_head[:, :, :, DH:VW], 1.0)

            # causal masks for the 4 diagonal offsets: keep q >= k
            masks = []
            for i in range(4):
                mask_t = const.tile([128, TQ], ST_DT, name=f"mask{i}")
                nc.vector.memset(mask_t, 1.0)
                nc.gpsimd.affine_select(
                    out=mask_t,
                    in_=mask_t,
                    compare_op=mybir.AluOpType.is_ge,
                    fill=0.0,
                    base=-(i * TKB),
                    pattern=[[1, TQ]],
                    channel_multiplier=-1,
                )
                masks.append(mask_t)

            # ---- Q/K projections: dest^T[m, t] = W_sel @ x^T (+bias) ----
            for w_sb, b_sb, dest in ((wq_sb, bq_sb, qt_sb), (wk_sb, bk_sb, kt_sb)):
                for mt in range(NPAIR):
                    pss = []
                    for nt in range(NQT):
                        ps_qk = ps512.tile([128, TQ], _f32, tag="ps512", name=f"psqk{nt}")
                        pss.append(ps_qk)
                    for kt in range(KD):
                        for nt in range(NQT):
                            nc.tensor.matmul(
                                pss[nt],
                                lhsT=mm(w_sb[:, kt, mt * 128 : (mt + 1) * 128]),
                                rhs=mm(xT_sb[:, kt, nt * TQ : (nt + 1) * TQ]),
                                start=(kt == 0),
                                stop=(kt == KD - 1),
                            )
                    for nt in range(NQT):
                        nc.vector.tensor_tensor(
                            out=dest[:, mt, nt * TQ : (nt + 1) * TQ],
                            in0=pss[nt],
                            in1=b_sb[:, mt : mt + 1].to_broadcast((128, TQ)),
                            op=mybir.AluOpType.add,
                        )

            # ---- V projection (per pair): V[t, hd] = x @ Wv_sel^T + bv ----
            for mt in range(NKT):
                for p in range(NPAIR):
                    psv = ps512.tile([128, 128], _f32, tag="ps512", name="psv")
                    for kt in range(KD):
                        nc.tensor.matmul(
                            psv,
                            lhsT=mm(xT_sb[:, kt, mt * 128 : (mt + 1) * 128]),
                            rhs=mm(wv_sb[:, kt, p * 128 : (p + 1) * 128]),
                            start=(kt == 0),
                            stop=(kt == KD - 1),
                        )
                    dst = v_by_head[:, mt, 2 * p : 2 * p + 2, 0:DH]
                    bvv = bvb_sb.rearrange("p (h c) -> p h c", c=VW)[
                        :, 2 * p : 2 * p + 2, 0:DH
                    ]
                    nc.vector.tensor_tensor(
                        out=dst,
                        in0=psv.rearrange("p (h c) -> p h c", c=DH),
                        in1=bvv,
                        op=mybir.AluOpType.add,
                    )

            # ---- attention per head pair ----
            for p in range(NPAIR):
                qA = qt_sb[0:64, p, :]
                qB = qt_sb[64:128, p, :]
                kA = kt_sb[0:64, p, :]
                kB = kt_sb[64:128, p, :]
                for qt in range(NQT):
                    nk = 4 * (qt + 1)
                    oA = ps512.tile([128, TQ], _f32, tag="ps512", name="oA")
                    oB = ps512.tile([128, TQ], _f32, tag="ps512", name="oB")
                    qsl = slice(qt * TQ, (qt + 1) * TQ)
                    pts = [None] * nk

                    def emit_qk(kt):
                        sab = ps1024.tile([128, 2, TQ], _f32, tag="sab", name="sab")
                        ksl = slice(kt * TKB, (kt + 1) * TKB)
                        nc.tensor.matmul(
                            sab[:, 0, :], lhsT=mm(kA[:, ksl]), rhs=mm(qA[:, qsl]),
                            start=True, stop=True,
                        )
                        nc.tensor.matmul(
                            sab[:, 1, :], lhsT=mm(kB[:, ksl]), rhs=mm(qB[:, qsl]),
                            start=True, stop=True,
                        )
                        pt = ptp.tile([128, 2, TQ], ST_DT, tag="pt", name="pt")
                        off = (kt - 4 * qt) * TKB if kt >= 4 * qt else 0
                        if off > 0:  # zero the fully-masked strip (cheap, GpSimd)
                            nc.gpsimd.memset(pt[:, :, 0:off], 0.0)
                        nc.scalar.activation(
                            out=pt[:, :, off:TQ], in_=sab[:, :, off:TQ],
                            func=mybir.ActivationFunctionType.Exp,
                            scale=0.125,
                        )
                        if kt >= 4 * qt:  # diagonal block: apply causal mask
                            mask_t = masks[kt - 4 * qt]
                            nc.vector.tensor_mul(pt[:, 0, :], pt[:, 0, :], mask_t)
                            nc.vector.tensor_mul(pt[:, 1, :], pt[:, 1, :], mask_t)
                        pts[kt] = pt

                    def emit_pv(kt):
                        st, sp = (kt == 0), (kt == nk - 1)
                        pt = pts[kt]
                        nc.tensor.matmul(
                            oA[0:VW, :],
                            lhsT=mm(v_sb[:, kt, (2 * p) * VW : (2 * p + 1) * VW]),
                            rhs=mm(pt[:, 0, :]),
                            start=st, stop=sp,
                        )
                        nc.tensor.matmul(
                            oB[0:VW, :],
                            lhsT=mm(v_sb[:, kt, (2 * p + 1) * VW : (2 * p + 2) * VW]),
                            rhs=mm(pt[:, 1, :]),
                            start=st, stop=sp,
                        )
                        pts[kt] = None

                    # software pipeline: PE stays one block ahead of ACT
                    for kt in range(nk):
                        emit_qk(kt)
                        if kt >= 1:
                            emit_pv(kt - 1)
                    emit_pv(nk - 1)

                    # normalize by the accumulated softmax sums (row 64).
                    # Reshape the 2x512 sums through DRAM into [64,16] so the
                    # (multi-pass) DVE reciprocal runs 64-partition-parallel.
                    sst = workp.tile([65, 2, TQ], _f32, tag="sst", name="sst")
                    nc.vector.tensor_copy(sst[64:65, 0, :], oA[64:65, :])
                    nc.vector.tensor_copy(sst[64:65, 1, :], oB[64:65, :])
                    nc.sync.dma_start(out=scr_d[p, qt], in_=sst[64:65, :, :])
                    sAB = workp.tile([64, 16], _f32, tag="sAB", name="sAB")
                    flat = scr_d[p, qt].rearrange("a b -> (a b)").rearrange(
                        "(p f) -> p f", p=64)
                    nc.sync.dma_start(out=sAB, in_=flat)
                    rAB = workp.tile([64, 16], _f32, tag="rAB", name="rAB")
                    nc.vector.reciprocal(out=rAB, in_=sAB)
                    flat2 = scr2_d[p, qt].rearrange("a b -> (a b)").rearrange(
                        "(p f) -> p f", p=64)
                    nc.sync.dma_start(out=flat2, in_=rAB)
                    rbA = workp.tile([64, TQ], _f32, tag="rbA", name="rbA")
                    rbB = workp.tile([64, TQ], _f32, tag="rbB", name="rbB")
                    rA = scr2_d[p, qt, 0, :]
                    rB = scr2_d[p, qt, 1, :]
                    nc.sync.dma_start(out=rbA, in_=bass.AP(
                        tensor=rA.tensor, offset=rA.offset,
                        ap=[[0, 64]] + list(rA.ap)))
                    nc.sync.dma_start(out=rbB, in_=bass.AP(
                        tensor=rB.tensor, offset=rB.offset,
                        ap=[[0, 64]] + list(rB.ap)))
                    nc.vector.tensor_mul(zt_sb[0:64, p, qsl], oA[0:64, :], rbA)
                    ztmp = workp.tile([64, TQ], ST_DT, tag="ztmp", name="ztmp")
                    nc.vector.tensor_mul(ztmp, oB[0:64, :], rbB)
                    nc.sync.dma_start(out=zt_sb[64:128, p, qsl], in_=ztmp)

            # ---- output projection: y^T[o, t] = Wo_sel^T... @ Z ----
            for mt in range(KD):
                psy = []
                for nt in range(NQT):
                    ps_y = ps512.tile([128, TQ], _f32, tag="ps512", name=f"psy{nt}")
                    psy.append(ps_y)
                for kt in range(NPAIR):
                    for nt in range(NQT):
                        nc.tensor.matmul(
                            psy[nt],
                            lhsT=mm(wo_sb[:, kt, mt * 128 : (mt + 1) * 128]),
                            rhs=mm(zt_sb[:, kt, nt * TQ : (nt + 1) * TQ]),
                            start=(kt == 0),
                            stop=(kt == NPAIR - 1),
                        )
                for nt in range(NQT):
                    yt = workp.tile([128, TQ], _f32, tag="yt", name="yt")
                    nc.vector.tensor_copy(yt, psy[nt])
                    nc.sync.dma_start(
                        out=yT_d[mt, :, nt * TQ : (nt + 1) * TQ], in_=yt
                    )

    if not nc.is_finalized():
        nc.finalize()
    return nc


_CACHE = {}


def get_program():
    key = MM_MODE
    if key not in _CACHE:
        _CACHE[key] = _build_program()
    return _CACHE[key]


def make_in_maps(x, wq, bq, wk, bk, wv, bv, wo, bo):
    _, np_dt, _ = _dts()
    x, wq, bq, wk, bk, wv, bv, wo, bo = (
        np.asarray(a, dtype=np.float32) for a in (x, wq, bq, wk, bk, wv, bv, wo, bo)
    )
    in_maps = []
    for core in range(8):
        b, hg = core // 2, core % 2
        sl = slice(hg * HPC * DH, (hg + 1) * HPC * DH)
        xT = np.ascontiguousarray(x[b].T).astype(np_dt).reshape(KD, 128, T)
        wqT = np.ascontiguousarray(wq[sl, :].T).astype(np_dt).reshape(KD, 128, HPC * DH)
        wkT = np.ascontiguousarray(wk[sl, :].T).astype(np_dt).reshape(KD, 128, HPC * DH)
        wvT = np.ascontiguousarray(wv[sl, :].T).astype(np_dt).reshape(KD, 128, HPC * DH)
        woT = np.ascontiguousarray(wo[:, sl].T).astype(np_dt).reshape(NPAIR, 128, D)
        bqT = np.ascontiguousarray(bq[sl].reshape(NPAIR, 128).T)
        bkT = np.ascontiguousarray(bk[sl].reshape(NPAIR, 128).T)
        bvb = np.zeros((HPC, VW), np.float32)
        bvb[:, :DH] = bv[sl].reshape(HPC, DH)
        bvb[:, DH] = 1.0
        in_maps.append(
            dict(xT=xT, wqT=wqT, wkT=wkT, wvT=wvT, woT=woT,
                 bqT=bqT, bkT=bkT, bvb=bvb.reshape(-1))
        )
    return in_maps


def assemble_output(results, bo):
    y = np.zeros((B, T, D), np.float32)
    for core in range(8):
        y[core // 2] += results[core]["yT"].reshape(D, T).T
    y += np.asarray(bo, np.float32)[None, None, :]
    return y


def kernel(**inputs):
    nc = get_program()
    in_maps = make_in_maps(**inputs)
    res = run_bass_kernel_spmd(nc, in_maps, core_ids=list(range(8)))
    return assemble_output(res.results, inputs["bo"])


if __name__ == "__main__":
    nc = get_program()
    print("program built OK")


# revision 14
# speedup vs baseline: 1.1425x; 1.0529x over previous
"""Causal self-attention Bass/Trainium2 kernel.

Problem: B=4, T=2048, D=768, NH=12 heads (dh=64), fp32 I/O.

Sharding (8 NeuronCores, no collectives):
  core = b * 2 + hg  for batch b in 0..3, head-group hg in 0..1.
  Each core computes 6 heads (hg*6 .. hg*6+5) of one batch:
    Q/K/V projections for its heads, causal attention, and the partial
    output projection y_part = Z_part @ Wo_part (row-split contraction).
  Host sums the two partial outputs per batch and adds bo.

Per-core kernel layout (everything transposed so the contraction dim is
on partitions; host pre-transposes, which is free):
  xT  [768, 2048]          QT/KT [384, 2048] (pairs of heads per 128-row tile)
  V'  [2048, 6*65]         (ones column appended per head -> softmax sums)
  S^T [128k, 512q] blocks, P = exp(S/8) (no max subtraction: |logits| < 10),
  O'  = V'.T @ P^T accumulated over k tiles -> row 64 holds softmax sums.
  Normalize by broadcasting 1/sums, assemble Z^T, then y^T = Wo_sel @ Z.
"""

import numpy as np
import ml_dtypes

import concourse.bass as bass
from concourse import bacc
import concourse.mybir as mybir
import concourse.tile as tile
from concourse.bass_utils import run_bass_kernel_spmd

B, T, D, NH, DH = 4, 2048, 768, 12, 64
HPC = 6          # heads per core
NPAIR = 3        # head pairs per core
TQ = 512         # query tile (free dim of S^T blocks)
NQT = T // TQ    # 4
TKB = 128        # key tile (partition dim of S^T blocks)
NKT = T // TKB   # 16
KD = D // 128    # 6 contraction tiles for the projections
VW = DH + 1      # 65: V plus ones column

# Matmul/storage dtype: "bf16", "fp32", or "fp32r".
MM_MODE = "bf16"

_f32 = mybir.dt.float32


def _dts():
    if MM_MODE == "bf16":
        return mybir.dt.bfloat16, ml_dtypes.bfloat16, False
    return _f32, np.float32, (MM_MODE == "fp32r")


def _build_program():
    ST_DT, _, use_r = _dts()

    def mm(ap):
        return ap.bitcast(mybir.dt.float32r) if use_r else ap

    nc = bacc.Bacc()
    xT_d = nc.dram_tensor("xT", [KD, 128, T], ST_DT, kind="ExternalInput")
    wq_d = nc.dram_tensor("wqT", [KD, 128, HPC * DH], ST_DT, kind="ExternalInput")
    wk_d = nc.dram_tensor("wkT", [KD, 128, HPC * DH], ST_DT, kind="ExternalInput")
    wv_d = nc.dram_tensor("wvT", [KD, 128, HPC * DH], ST_DT, kind="ExternalInput")
    wo_d = nc.dram_tensor("woT", [NPAIR, 128, D], ST_DT, kind="ExternalInput")
    bq_d = nc.dram_tensor("bqT", [128, NPAIR], _f32, kind="ExternalInput")
    bk_d = nc.dram_tensor("bkT", [128, NPAIR], _f32, kind="ExternalInput")
    bvb_d = nc.dram_tensor("bvb", [HPC * VW], _f32, kind="ExternalInput")
    # DRAM scratch used to broadcast the per-column softmax sums across
    # partitions (SBUF->DRAM->stride-0 DMA back; DVE cannot cross partitions)
    scr_d = nc.dram_tensor("rscratch", [NPAIR, NQT, 2, TQ], _f32, kind="Internal")
    scr2_d = nc.dram_tensor("rscratch2", [NPAIR, NQT, 2, TQ], _f32, kind="Internal")
    yT_d = nc.dram_tensor("yT", [KD, 128, T], _f32, kind="ExternalOutput")

    with tile.TileContext(nc) as tc:
        with (
            tc.tile_pool(name="const", bufs=1) as const,
            tc.tile_pool(name="ptp", bufs=3) as ptp,
            tc.tile_pool(name="workp", bufs=3) as workp,
            tc.tile_pool(name="ps512", bufs=4, space="PSUM") as ps512,
            tc.tile_pool(name="ps1024", bufs=2, space="PSUM") as ps1024,
        ):
            # ---- constants / persistent tensors ----
            xT_sb = const.tile([128, KD, T], ST_DT)
            wq_sb = const.tile([128, KD, HPC * DH], ST_DT)
            wk_sb = const.tile([128, KD, HPC * DH], ST_DT)
            wv_sb = const.tile([128, KD, HPC * DH], ST_DT)
            wo_sb = const.tile([128, NPAIR, D], ST_DT)
            bq_sb = const.tile([128, NPAIR], _f32)
            bk_sb = const.tile([128, NPAIR], _f32)
            bvb_sb = const.tile([128, HPC * VW], _f32)
            qt_sb = const.tile([128, NPAIR, T], ST_DT)
            kt_sb = const.tile([128, NPAIR, T], ST_DT)
            v_sb = const.tile([128, NKT, HPC * VW], ST_DT)
            zt_sb = const.tile([128, NPAIR, T], ST_DT)

            for kt in range(KD):
                # split xT tiles into 512-col chunks for finer DMA pipelining
                for c in range(4):
                    nc.sync.dma_start(
                        out=xT_sb[:, kt, c * TQ : (c + 1) * TQ],
                        in_=xT_d[kt][:, c * TQ : (c + 1) * TQ],
                    )
                nc.sync.dma_start(out=wq_sb[:, kt, :], in_=wq_d[kt])
                nc.sync.dma_start(out=wk_sb[:, kt, :], in_=wk_d[kt])
                nc.sync.dma_start(out=wv_sb[:, kt, :], in_=wv_d[kt])
            for kt in range(NPAIR):
                nc.sync.dma_start(out=wo_sb[:, kt, :], in_=wo_d[kt])
            nc.sync.dma_start(out=bq_sb, in_=bq_d[:, :])
            nc.sync.dma_start(out=bk_sb, in_=bk_d[:, :])
            bvb_ap = bvb_d[:]
            bvb_bcast = bass.AP(
                tensor=bvb_ap.tensor, offset=bvb_ap.offset,
                ap=[[0, 128]] + list(bvb_ap.ap),
            )
            nc.gpsimd.dma_start(out=bvb_sb, in_=bvb_bcast)

            # ones column of V' (softmax denominator accumulator)
            v_by_head = v_sb.rearrange("p m (h c) -> p m h c", c=VW)
            nc.vector.memset(v_by_head[:, :, :, DH:VW], 1.0)

            # causal masks for the 4 diagonal offsets: keep q >= k
            masks = []
            for i in range(4):
                mask_t = const.tile([128, TQ], ST_DT, name=f"mask{i}")
                nc.vector.memset(mask_t, 1.0)
                nc.gpsimd.affine_select(
                    out=mask_t,
                    in_=mask_t,
                    compare_op=mybir.AluOpType.is_ge,
                    fill=0.0,
                    base=-(i * TKB),
                    pattern=[[1, TQ]],
                    channel_multiplier=-1,
                )
                masks.append(mask_t)

            # ---- interleaved projections / attention / out-projection ----
            # All projection and out-projection matmul groups are emitted as
            # unit closures through a filler queue so they execute inside the
            # (otherwise ACT-bound) attention loops: this fills PE idle slots
            # and keeps the HAM clock-gate at full speed.
            from collections import deque

            queue = deque()          # pending (key, closure) units
            pending = {}             # key -> remaining unit count in queue
            emitted = set()

            def qk_group_units(which, mt, nt):
                w_sb, b_sb, dest = (
                    (wq_sb, bq_sb, qt_sb) if which == "q" else (wk_sb, bk_sb, kt_sb)
                )
                state = {}
                units = []
                for kt in range(KD):
                    def u(kt=kt):
                        if kt == 0:
                            state["ps"] = ps512.tile(
                                [128, TQ], _f32, tag="ps512", name="psg")
                        nc.tensor.matmul(
                            state["ps"],
                            lhsT=mm(w_sb[:, kt, mt * 128 : (mt + 1) * 128]),
                            rhs=mm(xT_sb[:, kt, nt * TQ : (nt + 1) * TQ]),
                            start=(kt == 0), stop=(kt == KD - 1),
                        )
                    units.append(u)
                def fin():
                    nc.vector.tensor_tensor(
                        out=dest[:, mt, nt * TQ : (nt + 1) * TQ],
                        in0=state["ps"],
                        in1=b_sb[:, mt : mt + 1].to_broadcast((128, TQ)),
                        op=mybir.AluOpType.add,
                    )
                units.append(fin)
                return units

            def v_group_units(mt):
                # all three pairs at once: rhs N=384
                state = {}
                units = []
                for kt in range(KD):
                    def u(kt=kt):
                        if kt == 0:
                            state["ps"] = ps512.tile(
                                [128, HPC * DH], _f32, tag="ps512", name="psg")
                        nc.tensor.matmul(
                            state["ps"],
                            lhsT=mm(xT_sb[:, kt, mt * 128 : (mt + 1) * 128]),
                            rhs=mm(wv_sb[:, kt, :]),
                            start=(kt == 0), stop=(kt == KD - 1),
                        )
                    units.append(u)
                def fin():
                    nc.vector.tensor_tensor(
                        out=v_by_head[:, mt, :, 0:DH],
                        in0=state["ps"].rearrange("p (h c) -> p h c", c=DH),
                        in1=bvb_sb.rearrange("p (h c) -> p h c", c=VW)[:, :, 0:DH],
                        op=mybir.AluOpType.add,
                    )
                units.append(fin)
                return units

            def o_group_units(mt, nt):
                state = {}
                units = []
                for kt in range(NPAIR):
                    def u(kt=kt):
                        if kt == 0:
                            state["ps"] = ps512.tile(
                                [128, TQ], _f32, tag="ps512", name="psg")
                        nc.tensor.matmul(
                            state["ps"],
                            lhsT=mm(wo_sb[:, kt, mt * 128 : (mt + 1) * 128]),
                            rhs=mm(zt_sb[:, kt, nt * TQ : (nt + 1) * TQ]),
                            start=(kt == 0), stop=(kt == NPAIR - 1),
                        )
                    units.append(u)
                def fin():
                    yt = workp.tile([128, TQ], _f32, tag="yt", name="yt")
                    nc.vector.tensor_copy(yt, state["ps"])
                    nc.sync.dma_start(
                        out=yT_d[mt, :, nt * TQ : (nt + 1) * TQ], in_=yt)
                units.append(fin)
                return units

            def units_for(key):
                kind = key[0]
                if kind == "q" or kind == "k":
                    return qk_group_units(kind, key[1], key[2])
                if kind == "v":
                    return v_group_units(key[1])
                return o_group_units(key[1], key[2])

            def push(key):
                if key in emitted:
                    return
                emitted.add(key)
                us = units_for(key)
                pending[key] = len(us)
                for u in us:
                    queue.append((key, u))

            def pop_unit():
                key, u = queue.popleft()
                u()
                pending[key] -= 1
                if pending[key] == 0:
                    del pending[key]

            def consume(n):
                for _ in range(n):
                    if queue:
                        pop_unit()

            def require(keys):
                # emit everything still queued for these groups right now
                for key in keys:
                    push(key)
                while any(pending.get(k, 0) > 0 for k in keys):
                    pop_unit()

            # queue pair-0 projections and all V in qt-demand order
            for nt in range(NQT):
                push(("k", 0, nt))
                push(("q", 0, nt))
                for mt in range(4 * nt, 4 * nt + 4):
                    push(("v", mt))

            # ---- attention per head pair ----
            for p in range(NPAIR):
                qA = qt_sb[0:64, p, :]
                qB = qt_sb[64:128, p, :]
                kA = kt_sb[0:64, p, :]
                kB = kt_sb[64:128, p, :]
                if p + 1 < NPAIR:  # queue next pair's Q/K projections
                    for nt in range(NQT):
                        push(("k", p + 1, nt))
                        push(("q", p + 1, nt))
                for qt in range(NQT):
                    nk = 4 * (qt + 1)
                    require(
                        [("k", p, nt) for nt in range(qt + 1)]
                        + [("q", p, qt)]
                        + [("v", mt) for mt in range(nk)]
                    )
                    oA = ps512.tile([128, TQ], _f32, tag="ps512", name="oA")
                    oB = ps512.tile([128, TQ], _f32, tag="ps512", name="oB")
                    qsl = slice(qt * TQ, (qt + 1) * TQ)
                    pts = [None] * nk

                    def emit_qk(kt):
                        sab = ps1024.tile([128, 2, TQ], _f32, tag="sab", name="sab")
                        ksl = slice(kt * TKB, (kt + 1) * TKB)
                        nc.tensor.matmul(
                            sab[:, 0, :], lhsT=mm(kA[:, ksl]), rhs=mm(qA[:, qsl]),
                            start=True, stop=True,
                        )
                        nc.tensor.matmul(
                            sab[:, 1, :], lhsT=mm(kB[:, ksl]), rhs=mm(qB[:, qsl]),
                            start=True, stop=True,
                        )
                        pt = ptp.tile([128, 2, TQ], ST_DT, tag="pt", name="pt")
                        off = (kt - 4 * qt) * TKB if kt >= 4 * qt else 0
                        if off > 0:  # zero the fully-masked strip (cheap, GpSimd)
                            nc.gpsimd.memset(pt[:, :, 0:off], 0.0)
                        nc.scalar.activation(
                            out=pt[:, :, off:TQ], in_=sab[:, :, off:TQ],
                            func=mybir.ActivationFunctionType.Exp,
                            scale=0.125,
                        )
                        if kt >= 4 * qt:  # diagonal block: apply causal mask
                            mask_t = masks[kt - 4 * qt]
                            nc.vector.tensor_mul(pt[:, 0, :], pt[:, 0, :], mask_t)
                            nc.vector.tensor_mul(pt[:, 1, :], pt[:, 1, :], mask_t)
                        pts[kt] = pt

                    def emit_pv(kt):
                        st, sp = (kt == 0), (kt == nk - 1)
                        pt = pts[kt]
                        nc.tensor.matmul(
                            oA[0:VW, :],
                            lhsT=mm(v_sb[:, kt, (2 * p) * VW : (2 * p + 1) * VW]),
                            rhs=mm(pt[:, 0, :]),
                            start=st, stop=sp,
                        )
                        nc.tensor.matmul(
                            oB[0:VW, :],
                            lhsT=mm(v_sb[:, kt, (2 * p + 1) * VW : (2 * p + 2) * VW]),
                            rhs=mm(pt[:, 1, :]),
                            start=st, stop=sp,
                        )
                        pts[kt] = None

                    # software pipeline: PE one block ahead of ACT; two filler
                    # units per iteration keep PE dense (HAM stays warm)
                    for kt in range(nk):
                        emit_qk(kt)
                        if kt >= 1:
                            emit_pv(kt - 1)
                        consume(2)
                    emit_pv(nk - 1)

                    # normalize by the accumulated softmax sums (row 64).
                    # Reshape the 2x512 sums through DRAM into [64,16] so the
                    # (multi-pass) DVE reciprocal runs 64-partition-parallel.
                    sst = workp.tile([65, 2, TQ], _f32, tag="sst", name="sst")
                    nc.vector.tensor_copy(sst[64:65, 0, :], oA[64:65, :])
                    nc.vector.tensor_copy(sst[64:65, 1, :], oB[64:65, :])
                    nc.sync.dma_start(out=scr_d[p, qt], in_=sst[64:65, :, :])
                    sAB = workp.tile([64, 16], _f32, tag="sAB", name="sAB")
                    flat = scr_d[p, qt].rearrange("a b -> (a b)").rearrange(
                        "(p f) -> p f", p=64)
                    nc.sync.dma_start(out=sAB, in_=flat)
                    rAB = workp.tile([64, 16], _f32, tag="rAB", name="rAB")
                    nc.vector.reciprocal(out=rAB, in_=sAB)
                    flat2 = scr2_d[p, qt].rearrange("a b -> (a b)").rearrange(
                        "(p f) -> p f", p=64)
                    nc.sync.dma_start(out=flat2, in_=rAB)
                    rbA = workp.tile([64, TQ], _f32, tag="rbA", name="rbA")
                    rbB = workp.tile([64, TQ], _f32, tag="rbB", name="rbB")
                    rA = scr2_d[p, qt, 0, :]
                    rB = scr2_d[p, qt, 1, :]
                    nc.sync.dma_start(out=rbA, in_=bass.AP(
                        tensor=rA.tensor, offset=rA.offset,
                        ap=[[0, 64]] + list(rA.ap)))
                    nc.sync.dma_start(out=rbB, in_=bass.AP(
                        tensor=rB.tensor, offset=rB.offset,
                        ap=[[0, 64]] + list(rB.ap)))
                    nc.vector.tensor_mul(zt_sb[0:64, p, qsl], oA[0:64, :], rbA)
                    ztmp = workp.tile([64, TQ], ST_DT, tag="ztmp", name="ztmp")
                    nc.vector.tensor_mul(ztmp, oB[0:64, :], rbB)
                    nc.sync.dma_start(out=zt_sb[64:128, p, qsl], in_=ztmp)

                    if p == NPAIR - 1:  # out-projection columns for this qt
                        for mt in range(KD):
                            push(("o", mt, qt))

            # drain whatever is left (tail of the out-projection)
            while queue:
                pop_unit()

    if not nc.is_finalized():
        nc.finalize()
    return nc


_CACHE = {}


def get_program():
    key = MM_MODE
    if key not in _CACHE:
        _CACHE[key] = _build_program()
    return _CACHE[key]


def make_in_maps(x, wq, bq, wk, bk, wv, bv, wo, bo):
    _, np_dt, _ = _dts()
    x, wq, bq, wk, bk, wv, bv, wo, bo = (
        np.asarray(a, dtype=np.float32) for a in (x, wq, bq, wk, bk, wv, bv, wo, bo)
    )
    in_maps = []
    for core in range(8):
        b, hg = core // 2, core % 2
        sl = slice(hg * HPC * DH, (hg + 1) * HPC * DH)
        xT = np.ascontiguousarray(x[b].T).astype(np_dt).reshape(KD, 128, T)
        wqT = np.ascontiguousarray(wq[sl, :].T).astype(np_dt).reshape(KD, 128, HPC * DH)
        wkT = np.ascontiguousarray(wk[sl, :].T).astype(np_dt).reshape(KD, 128, HPC * DH)
        wvT = np.ascontiguousarray(wv[sl, :].T).astype(np_dt).reshape(KD, 128, HPC * DH)
        woT = np.ascontiguousarray(wo[:, sl].T).astype(np_dt).reshape(NPAIR, 128, D)
        bqT = np.ascontiguousarray(bq[sl].reshape(NPAIR, 128).T)
        bkT = np.ascontiguousarray(bk[sl].reshape(NPAIR, 128).T)
        bvb = np.zeros((HPC, VW), np.float32)
        bvb[:, :DH] = bv[sl].reshape(HPC, DH)
        bvb[:, DH] = 1.0
        in_maps.append(
            dict(xT=xT, wqT=wqT, wkT=wkT, wvT=wvT, woT=woT,
                 bqT=bqT, bkT=bkT, bvb=bvb.reshape(-1))
        )
    return in_maps


def assemble_output(results, bo):
    y = np.zeros((B, T, D), np.float32)
    for core in range(8):
        y[core // 2] += results[core]["yT"].reshape(D, T).T
    y += np.asarray(bo, np.float32)[None, None, :]
    return y


def kernel(**inputs):
    nc = get_program()
    in_maps = make_in_maps(**inputs)
    res = run_bass_kernel_spmd(nc, in_maps, core_ids=list(range(8)))
    return assemble_output(res.results, inputs["bo"])


if __name__ == "__main__":
    nc = get_program()
    print("program built OK")


# revision 15
# speedup vs baseline: 1.2413x; 1.0865x over previous
"""Causal self-attention Bass/Trainium2 kernel.

Problem: B=4, T=2048, D=768, NH=12 heads (dh=64), fp32 I/O.

Sharding (8 NeuronCores, no collectives):
  core = b * 2 + hg  for batch b in 0..3, head-group hg in 0..1.
  Each core computes 6 heads (hg*6 .. hg*6+5) of one batch:
    Q/K/V projections for its heads, causal attention, and the partial
    output projection y_part = Z_part @ Wo_part (row-split contraction).
  Host sums the two partial outputs per batch and adds bo.

Per-core kernel layout (everything transposed so the contraction dim is
on partitions; host pre-transposes, which is free):
  xT  [768, 2048]          QT/KT [384, 2048] (pairs of heads per 128-row tile)
  V'  [2048, 6*65]         (ones column appended per head -> softmax sums)
  S^T [128k, 512q] blocks, P = exp(S/8) (no max subtraction: |logits| < 10),
  O'  = V'.T @ P^T accumulated over k tiles -> row 64 holds softmax sums.
  Normalize by broadcasting 1/sums, assemble Z^T, then y^T = Wo_sel @ Z.
"""

import numpy as np
import ml_dtypes

import concourse.bass as bass
from concourse import bacc
import concourse.mybir as mybir
import concourse.tile as tile
from concourse.bass_utils import run_bass_kernel_spmd

B, T, D, NH, DH = 4, 2048, 768, 12, 64
HPC = 6          # heads per core
NPAIR = 3        # head pairs per core
TQ = 512         # query tile (free dim of S^T blocks)
NQT = T // TQ    # 4
TKB = 128        # key tile (partition dim of S^T blocks)
NKT = T // TKB   # 16
KD = D // 128    # 6 contraction tiles for the projections
VW = DH + 1      # 65: V plus ones column

# Matmul/storage dtype: "bf16", "fp32", or "fp32r".
MM_MODE = "bf16"

_f32 = mybir.dt.float32


def _dts():
    if MM_MODE == "bf16":
        return mybir.dt.bfloat16, ml_dtypes.bfloat16, False
    return _f32, np.float32, (MM_MODE == "fp32r")


def _build_program():
    ST_DT, _, use_r = _dts()

    def mm(ap):
        return ap.bitcast(mybir.dt.float32r) if use_r else ap

    nc = bacc.Bacc()
    xT_d = nc.dram_tensor("xT", [KD, 128, T], ST_DT, kind="ExternalInput")
    wq_d = nc.dram_tensor("wqT", [KD, 128, HPC * DH], ST_DT, kind="ExternalInput")
    wk_d = nc.dram_tensor("wkT", [KD, 128, HPC * DH], ST_DT, kind="ExternalInput")
    wv_d = nc.dram_tensor("wvT", [KD, 128, HPC * DH], ST_DT, kind="ExternalInput")
    wo_d = nc.dram_tensor("woT", [NPAIR, 128, D], ST_DT, kind="ExternalInput")
    bq_d = nc.dram_tensor("bqT", [128, NPAIR], _f32, kind="ExternalInput")
    bk_d = nc.dram_tensor("bkT", [128, NPAIR], _f32, kind="ExternalInput")
    bvb_d = nc.dram_tensor("bvb", [HPC * VW], _f32, kind="ExternalInput")
    # DRAM scratch used to broadcast the per-column softmax sums across
    # partitions (SBUF->DRAM->stride-0 DMA back; DVE cannot cross partitions)
    scr_d = nc.dram_tensor("rscratch", [NPAIR, NQT, 2, TQ], _f32, kind="Internal")
    scr2_d = nc.dram_tensor("rscratch2", [NPAIR, NQT, 2, TQ], _f32, kind="Internal")
    yT_d = nc.dram_tensor("yT", [KD, 128, T], _f32, kind="ExternalOutput")

    with tile.TileContext(nc) as tc:
        with (
            tc.tile_pool(name="const", bufs=1) as const,
            tc.tile_pool(name="ptp", bufs=4) as ptp,
            tc.tile_pool(name="workp", bufs=3) as workp,
            tc.tile_pool(name="ps512", bufs=4, space="PSUM") as ps512,
            tc.tile_pool(name="ps1024", bufs=2, space="PSUM") as ps1024,
        ):
            # ---- constants / persistent tensors ----
            xT_sb = const.tile([128, KD, T], ST_DT)
            wq_sb = const.tile([128, KD, HPC * DH], ST_DT)
            wk_sb = const.tile([128, KD, HPC * DH], ST_DT)
            wv_sb = const.tile([128, KD, HPC * DH], ST_DT)
            wo_sb = const.tile([128, NPAIR, D], ST_DT)
            bq_sb = const.tile([128, NPAIR], _f32)
            bk_sb = const.tile([128, NPAIR], _f32)
            bvb_sb = const.tile([128, HPC * VW], _f32)
            qt_sb = const.tile([128, NPAIR, T], ST_DT)
            kt_sb = const.tile([128, NPAIR, T], ST_DT)
            v_sb = const.tile([128, NKT, HPC * VW], ST_DT)
            zt_sb = const.tile([128, NPAIR, T], ST_DT)

            for kt in range(KD):
                nc.sync.dma_start(out=wq_sb[:, kt, :], in_=wq_d[kt])
                nc.sync.dma_start(out=wk_sb[:, kt, :], in_=wk_d[kt])
                # split xT tiles into 512-col chunks for finer DMA pipelining
                for c in range(4):
                    nc.sync.dma_start(
                        out=xT_sb[:, kt, c * TQ : (c + 1) * TQ],
                        in_=xT_d[kt][:, c * TQ : (c + 1) * TQ],
                    )
            for kt in range(KD):
                nc.sync.dma_start(out=wv_sb[:, kt, :], in_=wv_d[kt])
            for kt in range(NPAIR):
                nc.sync.dma_start(out=wo_sb[:, kt, :], in_=wo_d[kt])
            nc.sync.dma_start(out=bq_sb, in_=bq_d[:, :])
            nc.sync.dma_start(out=bk_sb, in_=bk_d[:, :])
            bvb_ap = bvb_d[:]
            bvb_bcast = bass.AP(
                tensor=bvb_ap.tensor, offset=bvb_ap.offset,
                ap=[[0, 128]] + list(bvb_ap.ap),
            )
            nc.gpsimd.dma_start(out=bvb_sb, in_=bvb_bcast)

            # ones column of V' (softmax denominator accumulator)
            v_by_head = v_sb.rearrange("p m (h c) -> p m h c", c=VW)
            nc.vector.memset(v_by_head[:, :, :, DH:VW], 1.0)

            # one triangular causal mask tile: keep (col j) >= (row k)
            tri_sb = const.tile([128, TKB], ST_DT, name="tri")
            nc.vector.memset(tri_sb, 1.0)
            nc.gpsimd.affine_select(
                out=tri_sb,
                in_=tri_sb,
                compare_op=mybir.AluOpType.is_ge,
                fill=0.0,
                base=0,
                pattern=[[1, TKB]],
                channel_multiplier=-1,
            )

            # PE warm-up: dense dummy matmuls with no DMA deps keep the
            # tensor engine busy through the input DMA ramp so the HAM
            # clock-gate is at 2.4 GHz when real work arrives.
            dummy_sb = const.tile([128, TQ], ST_DT, name="dummy")
            nc.vector.memset(dummy_sb, 0.0)
            ps_warm = ps512.tile([128, TQ], _f32, tag="ps512", name="ps_warm")
            for _ in range(40):
                nc.tensor.matmul(
                    ps_warm, lhsT=mm(dummy_sb[:, 0:128]), rhs=mm(dummy_sb),
                    start=True, stop=True,
                )

            # ---- interleaved projections / attention / out-projection ----
            # All projection and out-projection matmul groups are emitted as
            # unit closures through a filler queue so they execute inside the
            # (otherwise ACT-bound) attention loops: this fills PE idle slots
            # and keeps the HAM clock-gate at full speed.
            from collections import deque

            queue = deque()          # pending (key, closure) units
            pending = {}             # key -> remaining unit count in queue
            emitted = set()

            def qk_group_units(which, mt, nt):
                w_sb, b_sb, dest = (
                    (wq_sb, bq_sb, qt_sb) if which == "q" else (wk_sb, bk_sb, kt_sb)
                )
                state = {}
                units = []
                for kt in range(KD):
                    def u(kt=kt):
                        if kt == 0:
                            state["ps"] = ps512.tile(
                                [128, TQ], _f32, tag="ps512", name="psg")
                        nc.tensor.matmul(
                            state["ps"],
                            lhsT=mm(w_sb[:, kt, mt * 128 : (mt + 1) * 128]),
                            rhs=mm(xT_sb[:, kt, nt * TQ : (nt + 1) * TQ]),
                            start=(kt == 0), stop=(kt == KD - 1),
                        )
                    units.append(u)
                def fin():
                    nc.vector.tensor_tensor(
                        out=dest[:, mt, nt * TQ : (nt + 1) * TQ],
                        in0=state["ps"],
                        in1=b_sb[:, mt : mt + 1].to_broadcast((128, TQ)),
                        op=mybir.AluOpType.add,
                    )
                units.append(fin)
                return units

            def v_group_units(mt):
                # all three pairs at once: rhs N=384
                state = {}
                units = []
                for kt in range(KD):
                    def u(kt=kt):
                        if kt == 0:
                            state["ps"] = ps512.tile(
                                [128, HPC * DH], _f32, tag="ps512", name="psg")
                        nc.tensor.matmul(
                            state["ps"],
                            lhsT=mm(xT_sb[:, kt, mt * 128 : (mt + 1) * 128]),
                            rhs=mm(wv_sb[:, kt, :]),
                            start=(kt == 0), stop=(kt == KD - 1),
                        )
                    units.append(u)
                def fin():
                    nc.vector.tensor_tensor(
                        out=v_by_head[:, mt, :, 0:DH],
                        in0=state["ps"].rearrange("p (h c) -> p h c", c=DH),
                        in1=bvb_sb.rearrange("p (h c) -> p h c", c=VW)[:, :, 0:DH],
                        op=mybir.AluOpType.add,
                    )
                units.append(fin)
                return units

            def o_group_units(mt, nt):
                state = {}
                units = []
                for kt in range(NPAIR):
                    def u(kt=kt):
                        if kt == 0:
                            state["ps"] = ps512.tile(
                                [128, TQ], _f32, tag="ps512", name="psg")
                        nc.tensor.matmul(
                            state["ps"],
                            lhsT=mm(wo_sb[:, kt, mt * 128 : (mt + 1) * 128]),
                            rhs=mm(zt_sb[:, kt, nt * TQ : (nt + 1) * TQ]),
                            start=(kt == 0), stop=(kt == NPAIR - 1),
                        )
                    units.append(u)
                def fin():
                    yt = workp.tile([128, TQ], _f32, tag="yt", name="yt")
                    nc.vector.tensor_copy(yt, state["ps"])
                    nc.sync.dma_start(
                        out=yT_d[mt, :, nt * TQ : (nt + 1) * TQ], in_=yt)
                units.append(fin)
                return units

            def units_for(key):
                kind = key[0]
                if kind == "q" or kind == "k":
                    return qk_group_units(kind, key[1], key[2])
                if kind == "v":
                    return v_group_units(key[1])
                return o_group_units(key[1], key[2])

            def push(key):
                if key in emitted:
                    return
                emitted.add(key)
                us = units_for(key)
                pending[key] = len(us)
                for u in us:
                    queue.append((key, u))

            def pop_unit():
                key, u = queue.popleft()
                u()
                pending[key] -= 1
                if pending[key] == 0:
                    del pending[key]

            def consume(n):
                for _ in range(n):
                    if queue:
                        pop_unit()

            def require(keys):
                # emit everything still queued for these groups right now
                for key in keys:
                    push(key)
                while any(pending.get(k, 0) > 0 for k in keys):
                    pop_unit()

            # queue pair-0 projections and all V in qt-demand order
            for nt in range(NQT):
                push(("k", 0, nt))
                push(("q", 0, nt))
                for mt in range(4 * nt, 4 * nt + 4):
                    push(("v", mt))

            # ---- attention per head pair ----
            for p in range(NPAIR):
                qA = qt_sb[0:64, p, :]
                qB = qt_sb[64:128, p, :]
                kA = kt_sb[0:64, p, :]
                kB = kt_sb[64:128, p, :]
                if p + 1 < NPAIR:  # queue next pair's Q/K projections
                    for nt in range(NQT):
                        push(("k", p + 1, nt))
                        push(("q", p + 1, nt))
                for qt in range(NQT):
                    nk = 4 * (qt + 1)
                    require(
                        [("k", p, nt) for nt in range(qt + 1)]
                        + [("q", p, qt)]
                        + [("v", mt) for mt in range(nk)]
                    )
                    oA = ps512.tile([128, TQ], _f32, tag="ps512", name="oA")
                    oB = ps512.tile([128, TQ], _f32, tag="ps512", name="oB")
                    qsl = slice(qt * TQ, (qt + 1) * TQ)
                    pts = [None] * nk

                    def emit_qk(kt):
                        sab = ps1024.tile([128, 2, TQ], _f32, tag="sab", name="sab")
                        ksl = slice(kt * TKB, (kt + 1) * TKB)
                        nc.tensor.matmul(
                            sab[:, 0, :], lhsT=mm(kA[:, ksl]), rhs=mm(qA[:, qsl]),
                            start=True, stop=True,
                        )
                        nc.tensor.matmul(
                            sab[:, 1, :], lhsT=mm(kB[:, ksl]), rhs=mm(qB[:, qsl]),
                            start=True, stop=True,
                        )
                        pt = ptp.tile([128, 2, TQ], ST_DT, tag="pt", name="pt")
                        off = (kt - 4 * qt) * TKB if kt >= 4 * qt else 0
                        if off > 0:  # zero the fully-masked strip (cheap, GpSimd)
                            nc.gpsimd.memset(pt[:, :, 0:off], 0.0)
                        nc.scalar.activation(
                            out=pt[:, :, off:TQ], in_=sab[:, :, off:TQ],
                            func=mybir.ActivationFunctionType.Exp,
                            scale=0.125,
                        )
                        if kt >= 4 * qt:  # diagonal block: mask the 128-col
                            dsl = slice(off, off + TKB)  # triangle only
                            nc.vector.tensor_mul(
                                pt[:, 0, dsl], pt[:, 0, dsl], tri_sb)
                            nc.vector.tensor_mul(
                                pt[:, 1, dsl], pt[:, 1, dsl], tri_sb)
                        pts[kt] = pt

                    def emit_pv(kt):
                        st, sp = (kt == 0), (kt == nk - 1)
                        pt = pts[kt]
                        nc.tensor.matmul(
                            oA[0:VW, :],
                            lhsT=mm(v_sb[:, kt, (2 * p) * VW : (2 * p + 1) * VW]),
                            rhs=mm(pt[:, 0, :]),
                            start=st, stop=sp,
                        )
                        nc.tensor.matmul(
                            oB[0:VW, :],
                            lhsT=mm(v_sb[:, kt, (2 * p + 1) * VW : (2 * p + 2) * VW]),
                            rhs=mm(pt[:, 1, :]),
                            start=st, stop=sp,
                        )
                        pts[kt] = None

                    # software pipeline: PE two blocks ahead of ACT; two
                    # filler units per iteration keep PE dense (HAM warm)
                    for kt in range(nk):
                        emit_qk(kt)
                        if kt >= 2:
                            emit_pv(kt - 2)
                        consume(2)
                    if nk >= 2:
                        emit_pv(nk - 2)
                    emit_pv(nk - 1)

                    # normalize by the accumulated softmax sums (row 64).
                    # Reshape the 2x512 sums through DRAM into [64,16] so the
                    # (multi-pass) DVE reciprocal runs 64-partition-parallel.
                    sst = workp.tile([65, 2, TQ], _f32, tag="sst", name="sst")
                    nc.vector.tensor_copy(sst[64:65, 0, :], oA[64:65, :])
                    nc.vector.tensor_copy(sst[64:65, 1, :], oB[64:65, :])
                    nc.sync.dma_start(out=scr_d[p, qt], in_=sst[64:65, :, :])
                    sAB = workp.tile([64, 16], _f32, tag="sAB", name="sAB")
                    flat = scr_d[p, qt].rearrange("a b -> (a b)").rearrange(
                        "(p f) -> p f", p=64)
                    nc.sync.dma_start(out=sAB, in_=flat)
                    rAB = workp.tile([64, 16], _f32, tag="rAB", name="rAB")
                    nc.vector.reciprocal(out=rAB, in_=sAB)
                    flat2 = scr2_d[p, qt].rearrange("a b -> (a b)").rearrange(
                        "(p f) -> p f", p=64)
                    nc.sync.dma_start(out=flat2, in_=rAB)
                    rbA = workp.tile([64, TQ], _f32, tag="rbA", name="rbA")
                    rbB = workp.tile([64, TQ], _f32, tag="rbB", name="rbB")
                    rA = scr2_d[p, qt, 0, :]
                    rB = scr2_d[p, qt, 1, :]
                    nc.sync.dma_start(out=rbA, in_=bass.AP(
                        tensor=rA.tensor, offset=rA.offset,
                        ap=[[0, 64]] + list(rA.ap)))
                    nc.sync.dma_start(out=rbB, in_=bass.AP(
                        tensor=rB.tensor, offset=rB.offset,
                        ap=[[0, 64]] + list(rB.ap)))
                    nc.vector.tensor_mul(zt_sb[0:64, p, qsl], oA[0:64, :], rbA)
                    ztmp = workp.tile([64, TQ], ST_DT, tag="ztmp", name="ztmp")
                    nc.vector.tensor_mul(ztmp, oB[0:64, :], rbB)
                    nc.sync.dma_start(out=zt_sb[64:128, p, qsl], in_=ztmp)

                    if p == NPAIR - 1:
                        # queue the PREVIOUS qt's out-projection columns now:
                        # its zt normalize chain has had a full qt to finish,
                        # so consuming these units never stalls PE
                        if qt >= 1:
                            for mt in range(KD):
                                push(("o", mt, qt - 1))

            # drain the tail of the out-projection
            for mt in range(KD):
                push(("o", mt, NQT - 1))
            while queue:
                pop_unit()

    if not nc.is_finalized():
        nc.finalize()
    return nc


_CACHE = {}


def get_program():
    key = MM_MODE
    if key not in _CACHE:
        _CACHE[key] = _build_program()
    return _CACHE[key]


def make_in_maps(x, wq, bq, wk, bk, wv, bv, wo, bo):
    _, np_dt, _ = _dts()
    x, wq, bq, wk, bk, wv, bv, wo, bo = (
        np.asarray(a, dtype=np.float32) for a in (x, wq, bq, wk, bk, wv, bv, wo, bo)
    )
    in_maps = []
    for core in range(8):
        b, hg = core // 2, core % 2
        sl = slice(hg * HPC * DH, (hg + 1) * HPC * DH)
        xT = np.ascontiguousarray(x[b].T).astype(np_dt).reshape(KD, 128, T)
        wqT = np.ascontiguousarray(wq[sl, :].T).astype(np_dt).reshape(KD, 128, HPC * DH)
        wkT = np.ascontiguousarray(wk[sl, :].T).astype(np_dt).reshape(KD, 128, HPC * DH)
        wvT = np.ascontiguousarray(wv[sl, :].T).astype(np_dt).reshape(KD, 128, HPC * DH)
        woT = np.ascontiguousarray(wo[:, sl].T).astype(np_dt).reshape(NPAIR, 128, D)
        bqT = np.ascontiguousarray(bq[sl].reshape(NPAIR, 128).T)
        bkT = np.ascontiguousarray(bk[sl].reshape(NPAIR, 128).T)
        bvb = np.zeros((HPC, VW), np.float32)
        bvb[:, :DH] = bv[sl].reshape(HPC, DH)
        bvb[:, DH] = 1.0
        in_maps.append(
            dict(xT=xT, wqT=wqT, wkT=wkT, wvT=wvT, woT=woT,
                 bqT=bqT, bkT=bkT, bvb=bvb.reshape(-1))
        )
    return in_maps


def assemble_output(results, bo):
    y = np.zeros((B, T, D), np.float32)
    for core in range(8):
        y[core // 2] += results[core]["yT"].reshape(D, T).T
    y += np.asarray(bo, np.float32)[None, None, :]
    return y


def kernel(**inputs):
    nc = get_program()
    in_maps = make_in_maps(**inputs)
    res = run_bass_kernel_spmd(nc, in_maps, core_ids=list(range(8)))
    return assemble_output(res.results, inputs["bo"])


if __name__ == "__main__":
    nc = get_program()
    print("program built OK")


# revision 16
# speedup vs baseline: 1.3040x; 1.0505x over previous
"""Causal self-attention Bass/Trainium2 kernel.

Problem: B=4, T=2048, D=768, NH=12 heads (dh=64), fp32 I/O.

Sharding (8 NeuronCores, no collectives):
  core = b * 2 + hg  for batch b in 0..3, head-group hg in 0..1.
  Each core computes 6 heads (hg*6 .. hg*6+5) of one batch:
    Q/K/V projections for its heads, causal attention, and the partial
    output projection y_part = Z_part @ Wo_part (row-split contraction).
  Host sums the two partial outputs per batch and adds bo.

Per-core kernel layout (everything transposed so the contraction dim is
on partitions; host pre-transposes, which is free):
  xT  [768, 2048]          QT/KT [384, 2048] (pairs of heads per 128-row tile)
  V'  [2048, 6*65]         (ones column appended per head -> softmax sums)
  S^T [128k, 512q] blocks, P = exp(S/8) (no max subtraction: |logits| < 10),
  O'  = V'.T @ P^T accumulated over k tiles -> row 64 holds softmax sums.
  Normalize by broadcasting 1/sums, assemble Z^T, then y^T = Wo_sel @ Z.
"""

import numpy as np
import ml_dtypes

import concourse.bass as bass
from concourse import bacc
import concourse.mybir as mybir
import concourse.tile as tile
from concourse.bass_utils import run_bass_kernel_spmd

B, T, D, NH, DH = 4, 2048, 768, 12, 64
HPC = 6          # heads per core
NPAIR = 3        # head pairs per core
TQ = 512         # query tile (free dim of S^T blocks)
NQT = T // TQ    # 4
TKB = 128        # key tile (partition dim of S^T blocks)
NKT = T // TKB   # 16
KD = D // 128    # 6 contraction tiles for the projections
VW = DH + 1      # 65: V plus ones column

# Matmul/storage dtype: "bf16", "fp32", or "fp32r".
MM_MODE = "bf16"

_f32 = mybir.dt.float32


def _dts():
    if MM_MODE == "bf16":
        return mybir.dt.bfloat16, ml_dtypes.bfloat16, False
    return _f32, np.float32, (MM_MODE == "fp32r")


def _build_program():
    ST_DT, _, use_r = _dts()

    def mm(ap):
        return ap.bitcast(mybir.dt.float32r) if use_r else ap

    nc = bacc.Bacc()
    xT_d = nc.dram_tensor("xT", [KD, 128, T], ST_DT, kind="ExternalInput")
    wq_d = nc.dram_tensor("wqT", [KD, 128, HPC * DH], ST_DT, kind="ExternalInput")
    wk_d = nc.dram_tensor("wkT", [KD, 128, HPC * DH], ST_DT, kind="ExternalInput")
    wv_d = nc.dram_tensor("wvT", [KD, 128, HPC * DH], ST_DT, kind="ExternalInput")
    wo_d = nc.dram_tensor("woT", [NPAIR, 128, D], ST_DT, kind="ExternalInput")
    bq_d = nc.dram_tensor("bqT", [128, NPAIR], _f32, kind="ExternalInput")
    bk_d = nc.dram_tensor("bkT", [128, NPAIR], _f32, kind="ExternalInput")
    bvb_d = nc.dram_tensor("bvb", [HPC * VW], _f32, kind="ExternalInput")
    # DRAM scratch used to broadcast the per-column softmax sums across
    # partitions (SBUF->DRAM->stride-0 DMA back; DVE cannot cross partitions)
    scr_d = nc.dram_tensor("rscratch", [NPAIR, NQT, 2, TQ], _f32, kind="Internal")
    scr2_d = nc.dram_tensor("rscratch2", [NPAIR, NQT, 2, TQ], _f32, kind="Internal")
    yT_d = nc.dram_tensor("yT", [KD, 128, T], _f32, kind="ExternalOutput")

    with tile.TileContext(nc) as tc:
        with (
            tc.tile_pool(name="const", bufs=1) as const,
            tc.tile_pool(name="ptp", bufs=4) as ptp,
            tc.tile_pool(name="workp", bufs=3) as workp,
            tc.tile_pool(name="ps512", bufs=4, space="PSUM") as ps512,
            tc.tile_pool(name="ps1024", bufs=2, space="PSUM") as ps1024,
        ):
            # ---- constants / persistent tensors ----
            xT_sb = const.tile([128, KD, T], ST_DT)
            wq_sb = const.tile([128, KD, HPC * DH], ST_DT)
            wk_sb = const.tile([128, KD, HPC * DH], ST_DT)
            wv_sb = const.tile([128, KD, HPC * DH], ST_DT)
            wo_sb = const.tile([128, NPAIR, D], ST_DT)
            bq_sb = const.tile([128, NPAIR], _f32)
            bk_sb = const.tile([128, NPAIR], _f32)
            bvb_sb = const.tile([128, HPC * VW], _f32)
            qt_sb = const.tile([128, NPAIR, T], ST_DT)
            kt_sb = const.tile([128, NPAIR, T], ST_DT)
            v_sb = const.tile([128, NKT, HPC * VW], ST_DT)
            zt_sb = const.tile([128, NPAIR, T], ST_DT)

            for kt in range(KD):
                nc.sync.dma_start(out=wq_sb[:, kt, :], in_=wq_d[kt])
                nc.sync.dma_start(out=wk_sb[:, kt, :], in_=wk_d[kt])
                # split xT tiles into 512-col chunks for finer DMA pipelining
                for c in range(4):
                    nc.sync.dma_start(
                        out=xT_sb[:, kt, c * TQ : (c + 1) * TQ],
                        in_=xT_d[kt][:, c * TQ : (c + 1) * TQ],
                    )
            for kt in range(KD):
                nc.sync.dma_start(out=wv_sb[:, kt, :], in_=wv_d[kt])
            for kt in range(NPAIR):
                nc.sync.dma_start(out=wo_sb[:, kt, :], in_=wo_d[kt])
            nc.sync.dma_start(out=bq_sb, in_=bq_d[:, :])
            nc.sync.dma_start(out=bk_sb, in_=bk_d[:, :])
            bvb_ap = bvb_d[:]
            bvb_bcast = bass.AP(
                tensor=bvb_ap.tensor, offset=bvb_ap.offset,
                ap=[[0, 128]] + list(bvb_ap.ap),
            )
            nc.gpsimd.dma_start(out=bvb_sb, in_=bvb_bcast)

            # ones column of V' (softmax denominator accumulator)
            v_by_head = v_sb.rearrange("p m (h c) -> p m h c", c=VW)
            nc.vector.memset(v_by_head[:, :, :, DH:VW], 1.0)

            # one triangular causal mask tile: keep (col j) >= (row k)
            tri_sb = const.tile([128, TKB], ST_DT, name="tri")
            nc.vector.memset(tri_sb, 1.0)
            nc.gpsimd.affine_select(
                out=tri_sb,
                in_=tri_sb,
                compare_op=mybir.AluOpType.is_ge,
                fill=0.0,
                base=0,
                pattern=[[1, TKB]],
                channel_multiplier=-1,
            )

            # PE warm-up: dense dummy matmuls with no DMA deps keep the
            # tensor engine busy through the input DMA ramp so the HAM
            # clock-gate is at 2.4 GHz when real work arrives.
            dummy_sb = const.tile([128, TQ], ST_DT, name="dummy")
            nc.vector.memset(dummy_sb, 0.0)
            ps_warm = ps512.tile([128, TQ], _f32, tag="ps512", name="ps_warm")
            for _ in range(40):
                nc.tensor.matmul(
                    ps_warm, lhsT=mm(dummy_sb[:, 0:128]), rhs=mm(dummy_sb),
                    start=True, stop=True,
                )

            # ---- interleaved projections / attention / out-projection ----
            # All projection and out-projection matmul groups are emitted as
            # unit closures through a filler queue so they execute inside the
            # (otherwise ACT-bound) attention loops: this fills PE idle slots
            # and keeps the HAM clock-gate at full speed.
            from collections import deque

            queue = deque()          # pending (key, closure) units
            pending = {}             # key -> remaining unit count in queue
            emitted = set()

            def qk_group_units(which, mt, nt):
                w_sb, b_sb, dest = (
                    (wq_sb, bq_sb, qt_sb) if which == "q" else (wk_sb, bk_sb, kt_sb)
                )
                state = {}
                units = []
                for kt in range(KD):
                    def u(kt=kt):
                        if kt == 0:
                            state["ps"] = ps512.tile(
                                [128, TQ], _f32, tag="ps512", name="psg")
                        nc.tensor.matmul(
                            state["ps"],
                            lhsT=mm(w_sb[:, kt, mt * 128 : (mt + 1) * 128]),
                            rhs=mm(xT_sb[:, kt, nt * TQ : (nt + 1) * TQ]),
                            start=(kt == 0), stop=(kt == KD - 1),
                        )
                    units.append(u)
                def fin():
                    nc.vector.tensor_tensor(
                        out=dest[:, mt, nt * TQ : (nt + 1) * TQ],
                        in0=state["ps"],
                        in1=b_sb[:, mt : mt + 1].to_broadcast((128, TQ)),
                        op=mybir.AluOpType.add,
                    )
                units.append(fin)
                return units

            def v_group_units(mt):
                # all three pairs at once: rhs N=384
                state = {}
                units = []
                for kt in range(KD):
                    def u(kt=kt):
                        if kt == 0:
                            state["ps"] = ps512.tile(
                                [128, HPC * DH], _f32, tag="ps512", name="psg")
                        nc.tensor.matmul(
                            state["ps"],
                            lhsT=mm(xT_sb[:, kt, mt * 128 : (mt + 1) * 128]),
                            rhs=mm(wv_sb[:, kt, :]),
                            start=(kt == 0), stop=(kt == KD - 1),
                        )
                    units.append(u)
                def fin():
                    nc.vector.tensor_tensor(
                        out=v_by_head[:, mt, :, 0:DH],
                        in0=state["ps"].rearrange("p (h c) -> p h c", c=DH),
                        in1=bvb_sb.rearrange("p (h c) -> p h c", c=VW)[:, :, 0:DH],
                        op=mybir.AluOpType.add,
                    )
                units.append(fin)
                return units

            def o_group_units(mt, nt):
                state = {}
                units = []
                for kt in range(NPAIR):
                    def u(kt=kt):
                        if kt == 0:
                            state["ps"] = ps512.tile(
                                [128, TQ], _f32, tag="ps512", name="psg")
                        nc.tensor.matmul(
                            state["ps"],
                            lhsT=mm(wo_sb[:, kt, mt * 128 : (mt + 1) * 128]),
                            rhs=mm(zt_sb[:, kt, nt * TQ : (nt + 1) * TQ]),
                            start=(kt == 0), stop=(kt == NPAIR - 1),
                        )
                    units.append(u)
                def fin():
                    yt = workp.tile([128, TQ], _f32, tag="yt", name="yt")
                    nc.vector.tensor_copy(yt, state["ps"])
                    nc.sync.dma_start(
                        out=yT_d[mt, :, nt * TQ : (nt + 1) * TQ], in_=yt)
                units.append(fin)
                return units

            def units_for(key):
                kind = key[0]
                if kind == "q" or kind == "k":
                    return qk_group_units(kind, key[1], key[2])
                if kind == "v":
                    return v_group_units(key[1])
                return o_group_units(key[1], key[2])

            def push(key):
                if key in emitted:
                    return
                emitted.add(key)
                us = units_for(key)
                pending[key] = len(us)
                for u in us:
                    queue.append((key, u))

            def pop_unit():
                key, u = queue.popleft()
                u()
                pending[key] -= 1
                if pending[key] == 0:
                    del pending[key]

            def consume(n):
                for _ in range(n):
                    if queue:
                        pop_unit()

            def require(keys):
                # emit everything still queued for these groups right now
                for key in keys:
                    push(key)
                while any(pending.get(k, 0) > 0 for k in keys):
                    pop_unit()

            # queue pair-0 projections and all V in qt-demand order
            for nt in range(NQT):
                push(("k", 0, nt))
                push(("q", 0, nt))
                for mt in range(4 * nt, 4 * nt + 4):
                    push(("v", mt))

            # ---- attention per head pair ----
            for p in range(NPAIR):
                qA = qt_sb[0:64, p, :]
                qB = qt_sb[64:128, p, :]
                kA = kt_sb[0:64, p, :]
                kB = kt_sb[64:128, p, :]
                if p + 1 < NPAIR:  # queue next pair's Q/K projections
                    for nt in range(NQT):
                        push(("k", p + 1, nt))
                        push(("q", p + 1, nt))
                for qt in range(NQT):
                    nk = 4 * (qt + 1)
                    require([("q", p, qt)])
                    oA = ps512.tile([128, TQ], _f32, tag="ps512", name="oA")
                    oB = ps512.tile([128, TQ], _f32, tag="ps512", name="oB")
                    qsl = slice(qt * TQ, (qt + 1) * TQ)
                    pts = [None] * nk

                    def emit_qk(kt):
                        sab = ps1024.tile([128, 2, TQ], _f32, tag="sab", name="sab")
                        ksl = slice(kt * TKB, (kt + 1) * TKB)
                        nc.tensor.matmul(
                            sab[:, 0, :], lhsT=mm(kA[:, ksl]), rhs=mm(qA[:, qsl]),
                            start=True, stop=True,
                        )
                        nc.tensor.matmul(
                            sab[:, 1, :], lhsT=mm(kB[:, ksl]), rhs=mm(qB[:, qsl]),
                            start=True, stop=True,
                        )
                        pt = ptp.tile([128, 2, TQ], ST_DT, tag="pt", name="pt")
                        off = (kt - 4 * qt) * TKB if kt >= 4 * qt else 0
                        if off > 0:  # zero the fully-masked strip (cheap, GpSimd)
                            nc.gpsimd.memset(pt[:, :, 0:off], 0.0)
                        nc.scalar.activation(
                            out=pt[:, :, off:TQ], in_=sab[:, :, off:TQ],
                            func=mybir.ActivationFunctionType.Exp,
                            scale=0.125,
                        )
                        if kt >= 4 * qt:  # diagonal block: mask the 128-col
                            dsl = slice(off, off + TKB)  # triangle only
                            nc.vector.tensor_mul(
                                pt[:, 0, dsl], pt[:, 0, dsl], tri_sb)
                            nc.vector.tensor_mul(
                                pt[:, 1, dsl], pt[:, 1, dsl], tri_sb)
                        pts[kt] = pt

                    def emit_pv(kt):
                        st, sp = (kt == 0), (kt == nk - 1)
                        pt = pts[kt]
                        nc.tensor.matmul(
                            oA[0:VW, :],
                            lhsT=mm(v_sb[:, kt, (2 * p) * VW : (2 * p + 1) * VW]),
                            rhs=mm(pt[:, 0, :]),
                            start=st, stop=sp,
                        )
                        nc.tensor.matmul(
                            oB[0:VW, :],
                            lhsT=mm(v_sb[:, kt, (2 * p + 1) * VW : (2 * p + 2) * VW]),
                            rhs=mm(pt[:, 1, :]),
                            start=st, stop=sp,
                        )
                        pts[kt] = None

                    # software pipeline: PE two blocks ahead of ACT; two
                    # filler units per iteration keep PE dense (HAM warm)
                    for kt in range(nk):
                        require([("k", p, kt // 4)])
                        emit_qk(kt)
                        if kt >= 2:
                            require([("v", kt - 2)])
                            emit_pv(kt - 2)
                        consume(3)
                    if nk >= 2:
                        require([("v", nk - 2)])
                        emit_pv(nk - 2)
                    require([("v", nk - 1)])
                    emit_pv(nk - 1)

                    # stage O' to SBUF immediately (frees both PSUM banks;
                    # a DVE copy costs the same regardless of partition count)
                    oAc = workp.tile([65, TQ], _f32, tag="oAc", name="oAc")
                    oBc = workp.tile([65, TQ], _f32, tag="oBc", name="oBc")
                    nc.vector.tensor_copy(oAc, oA[0:VW, :])
                    nc.vector.tensor_copy(oBc, oB[0:VW, :])
                    # normalize by the accumulated softmax sums (row 64).
                    # Reshape the 2x512 sums through DRAM into [64,16] so the
                    # (multi-pass) DVE reciprocal runs 64-partition-parallel.
                    nc.sync.dma_start(out=scr_d[p, qt, 0, :], in_=oAc[64:65, :])
                    nc.sync.dma_start(out=scr_d[p, qt, 1, :], in_=oBc[64:65, :])
                    sAB = workp.tile([64, 16], _f32, tag="sAB", name="sAB")
                    flat = scr_d[p, qt].rearrange("a b -> (a b)").rearrange(
                        "(p f) -> p f", p=64)
                    nc.sync.dma_start(out=sAB, in_=flat)
                    rAB = workp.tile([64, 16], _f32, tag="rAB", name="rAB")
                    nc.vector.reciprocal(out=rAB, in_=sAB)
                    flat2 = scr2_d[p, qt].rearrange("a b -> (a b)").rearrange(
                        "(p f) -> p f", p=64)
                    nc.sync.dma_start(out=flat2, in_=rAB)
                    rbA = workp.tile([64, TQ], _f32, tag="rbA", name="rbA")
                    rbB = workp.tile([64, TQ], _f32, tag="rbB", name="rbB")
                    rA = scr2_d[p, qt, 0, :]
                    rB = scr2_d[p, qt, 1, :]
                    nc.sync.dma_start(out=rbA, in_=bass.AP(
                        tensor=rA.tensor, offset=rA.offset,
                        ap=[[0, 64]] + list(rA.ap)))
                    nc.sync.dma_start(out=rbB, in_=bass.AP(
                        tensor=rB.tensor, offset=rB.offset,
                        ap=[[0, 64]] + list(rB.ap)))
                    nc.vector.tensor_mul(zt_sb[0:64, p, qsl], oAc[0:64, :], rbA)
                    ztmp = workp.tile([64, TQ], ST_DT, tag="ztmp", name="ztmp")
                    nc.vector.tensor_mul(ztmp, oBc[0:64, :], rbB)
                    nc.sync.dma_start(out=zt_sb[64:128, p, qsl], in_=ztmp)

                    if p == NPAIR - 1:
                        # queue the PREVIOUS qt's out-projection columns now:
                        # its zt normalize chain has had a full qt to finish,
                        # so consuming these units never stalls PE
                        if qt >= 1:
                            for mt in range(KD):
                                push(("o", mt, qt - 1))

            # drain the tail of the out-projection
            for mt in range(KD):
                push(("o", mt, NQT - 1))
            while queue:
                pop_unit()

    if not nc.is_finalized():
        nc.finalize()
    return nc


_CACHE = {}


def get_program():
    key = MM_MODE
    if key not in _CACHE:
        _CACHE[key] = _build_program()
    return _CACHE[key]


def make_in_maps(x, wq, bq, wk, bk, wv, bv, wo, bo):
    _, np_dt, _ = _dts()
    x, wq, bq, wk, bk, wv, bv, wo, bo = (
        np.asarray(a, dtype=np.float32) for a in (x, wq, bq, wk, bk, wv, bv, wo, bo)
    )
    in_maps = []
    for core in range(8):
        b, hg = core // 2, core % 2
        sl = slice(hg * HPC * DH, (hg + 1) * HPC * DH)
        xT = np.ascontiguousarray(x[b].T).astype(np_dt).reshape(KD, 128, T)
        wqT = np.ascontiguousarray(wq[sl, :].T).astype(np_dt).reshape(KD, 128, HPC * DH)
        wkT = np.ascontiguousarray(wk[sl, :].T).astype(np_dt).reshape(KD, 128, HPC * DH)
        wvT = np.ascontiguousarray(wv[sl, :].T).astype(np_dt).reshape(KD, 128, HPC * DH)
        woT = np.ascontiguousarray(wo[:, sl].T).astype(np_dt).reshape(NPAIR, 128, D)
        bqT = np.ascontiguousarray(bq[sl].reshape(NPAIR, 128).T)
        bkT = np.ascontiguousarray(bk[sl].reshape(NPAIR, 128).T)
        bvb = np.zeros((HPC, VW), np.float32)
        bvb[:, :DH] = bv[sl].reshape(HPC, DH)
        bvb[:, DH] = 1.0
        in_maps.append(
            dict(xT=xT, wqT=wqT, wkT=wkT, wvT=wvT, woT=woT,
                 bqT=bqT, bkT=bkT, bvb=bvb.reshape(-1))
        )
    return in_maps


def assemble_output(results, bo):
    y = np.zeros((B, T, D), np.float32)
    for core in range(8):
        y[core // 2] += results[core]["yT"].reshape(D, T).T
    y += np.asarray(bo, np.float32)[None, None, :]
    return y


def kernel(**inputs):
    nc = get_program()
    in_maps = make_in_maps(**inputs)
    res = run_bass_kernel_spmd(nc, in_maps, core_ids=list(range(8)))
    return assemble_output(res.results, inputs["bo"])


if __name__ == "__main__":
    nc = get_program()
    print("program built OK")
